# revision 31
# baseline (speedup 1.0000x reference)
"""Trainium2 Bass kernel: PreActBlock with DoReFa 4-bit quantization (sync-BN).

  out = conv3x3(q(relu(BN1(conv3x3(q(relu(BN0(x))), qw(w0))))), qw(w1)) + x

Design (8 cores, data-parallel over batch 16 -> 2 images/core):
 - Quantized activations are integers 0..15 and quantized weights odd integers
   -15..15 (x scale).  Both are exact in fp8e4 (e4m3) and the PE accumulates
   in fp32, so every conv is computed EXACTLY as integer sums (|S| < 2^20).
 - Weight quantization (tanh / absmax / round to 4-bit codes) is data
   independent, so it runs on the HOST inside kernel(): the device loads
   1.2MB of ready fp8 codes instead of 4.7MB fp32 + on-chip tanh/max
   pipeline.  The two derived scalars the device needs (eps' = eps/(M0/225)^2
   for BN1-on-integer-stats, and M1/225 for the output descale) ride in a
   small replicated hconst tensor.
 - fp8 DoubleRow matmuls: contraction K=256 per instruction via the
   [P, 2, ...] interleaved layout (2x PE throughput).
 - BN batch stats are all-reduced across the 8 cores (sync-BN semantics).
 - Critical-path focus: gpsimd's queue carries ONLY stats work and the two
   AllReduce triggers (collectives must fire the moment their input DMA
   lands).  x is loaded in 8 chunks over the sync/scalar/vector/tensor
   queues; stats are computed per-chunk on three engines in parallel
   (scalar: img0 sum-of-squares, gpsimd: img0 sum, vector: img1 bn_stats).
 - BN coefficient chains are short: EPS folds into the Sqrt activation bias,
   rsqrt = vector.reciprocal + scalar Sqrt (no Newton; tolerance budget is
   ~2e-2 and quantization rounding dominates), and the BN1 chain runs
   directly on integer stats (scale folded into eps').
 - Act layout [P, row, ki, 64]: ki innermost (stride 64 so DoubleRow APs are
   legal) keeps sub-tile dependency intervals row-tight, so conv matmuls can
   start as soon as the first quantized rows land.
 - x and the intermediate conv0 output S live entirely in SBUF (no DRAM
   spill/reload), so quant1 -> conv1 starts right after the second AllReduce.
"""
import os
import sys

sys.path.insert(0, "/opt/trn_rl_repo")

import ml_dtypes
import numpy as np

import concourse.bacc as bacc
import concourse.mybir as mybir
from concourse import tile
from concourse import bass_utils

F32 = mybir.dt.float32
FP8 = mybir.dt.float8e4
I8 = mybir.dt.int8
AX = mybir.AxisListType
OP = mybir.AluOpType
AF = mybir.ActivationFunctionType
PM = mybir.MatmulPerfMode

P = 128
N_CORES = 8
IMG = 2              # images per core
H = 56
ROWS = 116           # 2 images x (1 pad + 56 + 1 pad) rows
CW = 64              # padded column stride of act rows (ki stride, %16 == 0)
CNT = 50176.0        # global BN count: 16 * 56 * 56
EPS = 1e-5

# per-image 9-row output windows (junk boundary rows 57/58 never computed)
WINDOWS = ([(1 + 9 * k, 9) for k in range(6)] + [(55, 2)] +
           [(59 + 9 * k, 9) for k in range(6)] + [(113, 2)])
GROUPS = [WINDOWS[0:2], WINDOWS[2:6], WINDOWS[6:10], WINDOWS[10:14]]
# tap order: full-width tap (dy=0,dx=1) first so start=True covers all columns
TAPS = [(0, 1), (0, 0), (0, 2), (1, 0), (1, 1), (1, 2), (2, 0), (2, 1), (2, 2)]

# x chunk order: img1 early (vector's bn_stats tail is longest), interleaved
# with img0 (scalar + gpsimd).  Only gpsimd/sync/scalar queues can trigger
# DMAs; gpsimd fires the two first-to-process chunks at the very top of its
# queue (its stats ops follow them in program order).
X_ORDER = [(1, 0, 0), (0, 0, 0), (1, 0, 1), (0, 0, 1),
           (1, 1, 0), (0, 1, 0), (1, 1, 1), (0, 1, 1)]


def _runs(r0, nr):
    """Interior row-runs of a window: (logical_row, nrows, img, h0)."""
    out = []
    for lo, hi, img, base in ((1, 56, 0, 1), (59, 114, 1, 59)):
        a, b = max(r0, lo), min(r0 + nr - 1, hi)
        if a <= b:
            out.append((a, b - a + 1, img, a - base))
    return out


def build():
    nc = bacc.Bacc("TRN2", target_bir_lowering=False, debug=False,
                   enable_asserts=False, num_devices=N_CORES)

    x_d = nc.dram_tensor("x", [IMG, 256, H, H], F32, kind="ExternalInput")
    # host-quantized weight codes, [ci_lo, tap, ki, co] fp8 (odd ints)
    wq_d = [nc.dram_tensor("wq0", [P, 9, 2, 256], FP8, kind="ExternalInput"),
            nc.dram_tensor("wq1", [P, 9, 2, 256], FP8, kind="ExternalInput")]
    g_d = [nc.dram_tensor("bn0_gamma", [256], F32, kind="ExternalInput"),
           nc.dram_tensor("bn1_gamma", [256], F32, kind="ExternalInput")]
    b_d = [nc.dram_tensor("bn0_beta", [256], F32, kind="ExternalInput"),
           nc.dram_tensor("bn1_beta", [256], F32, kind="ExternalInput")]
    # replicated host constants:
    #   col0 = EPS, col1 = eps' = EPS/(M0/225)^2, col2 = M1/225
    hc_d = nc.dram_tensor("hconst", [P, 3], F32, kind="ExternalInput")
    out_d = nc.dram_tensor("out", [IMG, 256, H, H], F32, kind="ExternalOutput")

    xv = x_d.ap().rearrange("n c h w -> c n h w")       # [256, 2, 56, 56]
    ov = out_d.ap().rearrange("n c h w -> c n h w")

    with tile.TileContext(nc) as tc:
        with tc.tile_pool(name="act", bufs=1) as actp, \
             tc.tile_pool(name="wtp", bufs=1) as wtp, \
             tc.tile_pool(name="qt", bufs=3) as qtp, \
             tc.tile_pool(name="run", bufs=6) as runp, \
             tc.tile_pool(name="st", bufs=1) as stp, \
             tc.tile_pool(name="ps", bufs=8, space="PSUM") as psp, \
             tc.tile_pool(name="dram", bufs=1, space="DRAM") as drp:

            # ---------- static tiles ----------
            # act layout: [P, row, ki, CW]; image cols 0..55, cols 56..63 are
            # never-read filler that pads the ki stride to 64.
            act0 = actp.tile([P, ROWS, 2, CW], FP8, name="act0")
            act1 = actp.tile([P, ROWS, 2, CW], FP8, name="act1")
            actv = [act0.rearrange("p r k c -> p k r c"),
                    act1.rearrange("p r k c -> p k r c")]
            # quantized weight codes, [ci_lo, tap, ki, co] fp8
            wT = [wtp.tile([P, 9, 2, 256], FP8, name=f"w{v}T") for v in range(2)]
            # full x and conv0-integer-output S, SBUF-resident per co half
            x_sb = [actp.tile([P, IMG, H, H], F32, name=f"x_sb_{c}")
                    for c in range(2)]
            s_sb = [actp.tile([P, IMG, H, H], F32, name=f"s_sb_{c}")
                    for c in range(2)]
            x_fl = [t.rearrange("p i h w -> p (i h w)") for t in x_sb]
            s_fl = [t.rearrange("p i h w -> p (i h w)") for t in s_sb]
            ar_in = [drp.tile([P, 4], F32, name=f"ar_in_{i}") for i in range(2)]
            # AllGather output: rank-major blocks [r][p][c] in DRAM
            ar_out = [drp.tile([N_CORES, P, 4], F32, name=f"ar_out_{i}")
                      for i in range(2)]
            ag_sb = [stp.tile([P, 8, 4], F32, name=f"ag_sb_{i}")
                     for i in range(2)]

            # stats / small vectors
            xbn = [stp.tile([P, 8, 6], F32, name=f"xbn_{c}") for c in range(2)]
            gsx = [stp.tile([P, 2], F32, name=f"gsx_{c}") for c in range(2)]
            sqx = [stp.tile([P, 2], F32, name=f"sqx_{c}") for c in range(2)]
            sumS = [stp.tile([P, 16], F32, name=f"sumS_{c}") for c in range(2)]
            sqS = [stp.tile([P, 16], F32, name=f"sqS_{c}") for c in range(2)]
            gvec = [stp.tile([P, 2], F32, name=f"g{v}") for v in range(2)]
            bvec = [stp.tile([P, 2], F32, name=f"b{v}") for v in range(2)]
            avec = [stp.tile([P, 2], F32, name=f"a{v}") for v in range(2)]
            bbvec = [stp.tile([P, 2], F32, name=f"bb{v}") for v in range(2)]
            hc = stp.tile([P, 3], F32, name="hc")
            # gpsimd warm-keeping scratch (see gwarm scope below)
            gwarm = stp.tile([P, 8], F32, name="gwarm")
            pk = [stp.tile([P, 4], F32, name=f"pk_{i}") for i in range(2)]
            gpk = [stp.tile([P, 4], F32, name=f"gpk_{i}") for i in range(2)]

            def vtile(name, w=1):
                return stp.tile([P, w], F32, name=name, tag="vtmp", bufs=8,
                                padded_shape=[P, 4])

            # ---------- x load: paced triggers on the scalar queue ----------
            # The DMA engines fair-share bandwidth over all pending
            # transfers, so triggering everything up front makes every chunk
            # land at the END of the load.  Instead the scalar queue
            # interleaves triggers with its own per-chunk stats passes, so
            # chunks land in processing order and stats pipeline with the
            # load.  img0 sum-of-squares on scalar; img0 sum + img1 bn_stats
            # on vector.
            def xtrig(img, blk, c):
                h0 = blk * 28
                nc.scalar.dma_start(x_sb[c][:, img, h0:h0 + 28, :],
                                    xv[c * P:(c + 1) * P, img, h0:h0 + 28, :])

            def sq_pass(blk, c):
                h0 = blk * 28
                nc.scalar.activation(
                    s_sb[c][:, 0, h0:h0 + 28, :],
                    x_sb[c][:, 0, h0:h0 + 28, :], AF.Square,
                    accum_out=sqx[c][:, blk:blk + 1])

            def sum_pass(blk, c):
                h0 = blk * 28
                nc.vector.tensor_scalar(
                    s_sb[c][:, 1, h0:h0 + 28, :],
                    x_sb[c][:, 0, h0:h0 + 28, :], 1.0, 0.0,
                    OP.mult, OP.add, accum_out=gsx[c][:, blk:blk + 1])

            def bn_pass(blk, c):
                h0 = blk * 28
                fl = x_fl[c][:, (H + h0) * H:(H + h0 + 28) * H]
                k = blk * 4
                for cc in range(4):
                    nc.vector.bn_stats(xbn[c][:, k + cc, :],
                                       fl[:, cc * 392:(cc + 1) * 392])

            with nc.named_scope("stats0"):
                xtrig(1, 0, 0)
                xtrig(0, 0, 0)
                xtrig(1, 0, 1)
                xtrig(0, 0, 1)
                sq_pass(0, 0)          # scalar queue: waits chunk (0,0,0)
                xtrig(1, 1, 0)
                xtrig(0, 1, 0)
                sq_pass(0, 1)
                xtrig(0, 1, 1)         # img0/c1 lands 7th: its Square pass
                xtrig(1, 1, 1)         # overlaps the final img1 bn_stats
                sq_pass(1, 0)
                sq_pass(1, 1)
                # pre-touch Sqrt while scalar is idle: loads its activation
                # table now instead of on the post-AllGather critical path
                # (the ag-back half-DMA below blocks the scalar queue until
                # the collective completes)
                twarm = vtile("sqrt_warm")
                nc.scalar.activation(twarm[:], sqx[1][:, 1:2], AF.Sqrt)
                # vector consumes in landing order
                bn_pass(0, 0)
                sum_pass(0, 0)
                bn_pass(0, 1)
                sum_pass(0, 1)
                bn_pass(1, 0)
                sum_pass(1, 0)
                sum_pass(1, 1)
                bn_pass(1, 1)

            # weight-code + param loads: scalar queue, after the x pacers
            for v in range(2):
                nc.scalar.dma_start(wT[v][:], wq_d[v].ap()[:])
            for v in range(2):
                for c in range(2):
                    nc.scalar.dma_start(gvec[v][:, c:c + 1],
                                        g_d[v].ap()[c * P:(c + 1) * P])
                    nc.scalar.dma_start(bvec[v][:, c:c + 1],
                                        b_d[v].ap()[c * P:(c + 1) * P])
            nc.scalar.dma_start(hc[:], hc_d.ap()[:])

            # ---------- act pad-row zeroing (vector, overlaps x DMA) --------
            with nc.named_scope("memset"):
                for t in (act0, act1):
                    for r in (0, 57, 58, 115):
                        nc.vector.memset(t[:, r, :, 0:H], 0.0)

            with nc.named_scope("gwarm"):
                for k, (img, blk, c) in enumerate(
                        [(1, 0, 0), (0, 0, 1), (1, 1, 0), (0, 1, 0),
                         (0, 1, 1), (1, 1, 1)]):
                    nc.gpsimd.dma_start(
                        gwarm[:, k:k + 1],
                        x_sb[c][:, img, blk * 28 + 27, 0:1])

            with nc.named_scope("stats0c"):
                for c in range(2):
                    mv = stp.tile([P, 2], F32, name=f"mv0_{c}")
                    nc.vector.bn_aggr(mv[:], xbn[c][:, 0:8, :])
                    sv_ = vtile(f"sv_{c}")
                    nc.vector.scalar_tensor_tensor(
                        sv_[:], mv[:, 0:1], 3136.0, gsx[c][:, 0:1],
                        OP.mult, OP.add)
                    nc.vector.tensor_add(pk[0][:, c:c + 1], sv_[:],
                                         gsx[c][:, 1:2])
                    m2 = vtile(f"xm2_{c}")
                    nc.vector.tensor_mul(m2[:], mv[:, 0:1], mv[:, 0:1])
                    vp = vtile(f"xvp_{c}")
                    nc.vector.tensor_add(vp[:], mv[:, 1:2], m2[:])
                    qv_ = vtile(f"qv_{c}")
                    nc.vector.scalar_tensor_tensor(
                        qv_[:], vp[:], 3136.0, sqx[c][:, 0:1],
                        OP.mult, OP.add)
                    nc.vector.tensor_add(pk[0][:, 2 + c:3 + c], qv_[:],
                                         sqx[c][:, 1:2])
                nc.gpsimd.dma_start(ar_in[0][:], pk[0][:])
                nc.gpsimd.collective_compute(
                    "AllGather", OP.bypass,
                    replica_groups=[list(range(N_CORES))],
                    ins=[ar_in[0].opt()], outs=[ar_out[0].opt()])
                # gather-back split across two queues: the [P,8,4] view is
                # 16B-per-descriptor strided, so halving it in parallel
                # roughly halves the flight time
                nc.sync.dma_start(
                    ag_sb[0][:, 0:4, :],
                    ar_out[0].rearrange("r p c -> p r c")[:, 0:4, :])
                nc.scalar.dma_start(
                    ag_sb[0][:, 4:8, :],
                    ar_out[0].rearrange("r p c -> p r c")[:, 4:8, :])
                nc.vector.tensor_reduce(
                    gpk[0][:], ag_sb[0].rearrange("p r c -> p c r")[:],
                    AX.X, OP.add)

            # ---------- BN coefficients: z = a*S + b on [P, 2] ----------
            # a = 15*gamma*rsqrt(var + eps), b = 15*beta - mean*a.
            # For v=1 the stats are raw integer-S stats and eps_bias = eps'
            # (scale folded in on the host), so the chain is identical.
            def bn_coeffs(v, eps_bias):
                me = vtile(f"me{v}", 4)
                nc.vector.tensor_scalar(me[:], gpk[v][:, 0:4], 1.0 / CNT,
                                        None, OP.mult)
                mean, ex2 = me[:, 0:2], me[:, 2:4]
                m2 = vtile(f"m2{v}", 2)
                nc.vector.tensor_mul(m2[:], mean, mean)
                vpe = vtile(f"vp{v}", 2)
                nc.vector.tensor_sub(vpe[:], ex2, m2[:])
                y = vtile(f"y{v}", 2)
                nc.scalar.activation(y[:], vpe[:], AF.Sqrt, bias=eps_bias)
                rr = vtile(f"rr{v}", 2)
                nc.vector.reciprocal(rr[:], y[:])
                grs = vtile(f"gr{v}", 2)
                nc.vector.tensor_mul(grs[:], gvec[v][:], rr[:])
                nc.vector.tensor_scalar(avec[v][:], grs[:], 15.0, None,
                                        OP.mult)
                mg = vtile(f"mg{v}", 2)
                nc.vector.tensor_mul(mg[:], mean, avec[v][:])
                nc.vector.scalar_tensor_tensor(
                    bbvec[v][:], bvec[v][:], 15.0, mg[:],
                    OP.mult, OP.subtract)

            with nc.named_scope("coeffs0"):
                bn_coeffs(0, hc[:, 0:1])

            # ---------- quantize pipeline ----------
            # stage1: int8(relu(a*x+b)) on scalar (RNE cast rounds)
            # stage2: vector min(.,15) -> fp8 act codes
            def quantize_block(src_t, act_t, c, img, h0, nr, v, names):
                lr = img * 58 + 1 + h0   # logical row
                src_ch = src_t[:, img, h0:h0 + nr, :]
                u = qtp.tile([P, nr, H], I8, name=names + "u", tag="qu",
                             bufs=4, padded_shape=[P, 28, H])
                nc.scalar.activation(u[:], src_ch, AF.Relu,
                                     bias=bbvec[v][:, c:c + 1],
                                     scale=avec[v][:, c:c + 1])
                nc.vector.tensor_scalar(act_t[:, lr:lr + nr, c, 0:H], u[:],
                                        15.0, None, OP.min)

            def quant_phase(src, act_t, v, tag):
                # first block = exactly the 11 input rows of conv window 1,
                # so the first matmul fires as soon as possible after the
                # AllReduce
                for c in range(2):
                    quantize_block(src[c], act_t, c, 0, 0, 11, v,
                                   f"{tag}_{c}00a")
                for c in range(2):
                    quantize_block(src[c], act_t, c, 0, 11, 17, v,
                                   f"{tag}_{c}00b")
                for img in range(IMG):
                    for blk in range(2):
                        if img == 0 and blk == 0:
                            continue
                        for c in range(2):
                            quantize_block(src[c], act_t, c, img, blk * 28,
                                           28, v, f"{tag}_{c}{img}{blk}")

            with nc.named_scope("quant0"):
                quant_phase(x_sb, act0, 0, "q0")

            # ---------- conv (shared), fp8 DoubleRow, K=256 per matmul -------
            def conv(v, epilogue):
                av = actv[v]
                for gi, grp in enumerate(GROUPS):
                    for co in range(2):
                        psums = []
                        for wi, (r0, nr) in enumerate(grp):
                            ps = psp.tile([P, nr, H], F32,
                                          name=f"ps{v}_{gi}_{co}_{wi}",
                                          tag="psw", padded_shape=[P, 9, H])
                            psums.append(ps)
                        for ti, (dy, dx) in enumerate(TAPS):
                            tap = dy * 3 + dx
                            wlo, whi = max(0, 1 - dx), min(H, H + 1 - dx)
                            jlo = max(0, dx - 1)
                            lhsT = wT[v][:, tap, :, co * P:(co + 1) * P]
                            first = ti == 0
                            last = ti == 8
                            for wi, (r0, nr) in enumerate(grp):
                                rows = slice(r0 + dy - 1, r0 + dy - 1 + nr)
                                rhs = av[:, :, rows, jlo:jlo + whi - wlo]
                                if dx == 1:
                                    out = psums[wi][:, :, :]
                                else:
                                    out = psums[wi][:, :, wlo:whi]
                                nc.tensor.matmul(out, lhsT, rhs,
                                                 start=first, stop=last,
                                                 perf_mode=PM.DoubleRow)
                        for wi, (r0, nr) in enumerate(grp):
                            epilogue(co, r0, nr, psums[wi])

            # ---------- conv0 epilogue: S -> SBUF + interior sums ----------
            # sum on scalar (psum drain w/ accum), square-sum on gpsimd
            slot_idx = [0, 0]

            def epi0(co, r0, nr, ps):
                psf = ps.rearrange("p r c -> p (r c)")
                for (rl, n, img, h0) in _runs(r0, nr):
                    sl = psf[:, (rl - r0) * H:(rl - r0 + n) * H]
                    dst = s_fl[co][:, (img * H + h0) * H:(img * H + h0 + n) * H]
                    k = slot_idx[co]
                    slot_idx[co] += 1
                    nc.scalar.activation(dst, sl, AF.Identity,
                                         accum_out=sumS[co][:, k:k + 1])
                    sq = runp.tile([P, n * H], F32, name=f"sq_{co}_{rl}",
                                   tag="sq", bufs=2, padded_shape=[P, 9 * H])
                    nc.vector.scalar_tensor_tensor(
                        sq[:], dst, 0.0, dst, OP.bypass, OP.mult,
                        accum_out=sqS[co][:, k:k + 1])

            with nc.named_scope("conv0"):
                conv(0, epi0)

            # ---------- BN1 stats + AR ----------
            with nc.named_scope("stats1"):
                for c in range(2):
                    ns = slot_idx[c]
                    nc.vector.tensor_reduce(pk[1][:, c:c + 1],
                                            sumS[c][:, 0:ns], AX.X, OP.add)
                    nc.vector.tensor_reduce(pk[1][:, 2 + c:3 + c],
                                            sqS[c][:, 0:ns], AX.X, OP.add)
                nc.gpsimd.dma_start(ar_in[1][:], pk[1][:])
                nc.gpsimd.collective_compute(
                    "AllGather", OP.bypass,
                    replica_groups=[list(range(N_CORES))],
                    ins=[ar_in[1].opt()], outs=[ar_out[1].opt()])
                # gather-back split across two queues: the [P,8,4] view is
                # 16B-per-descriptor strided, so halving it in parallel
                # roughly halves the flight time
                nc.sync.dma_start(
                    ag_sb[1][:, 0:4, :],
                    ar_out[1].rearrange("r p c -> p r c")[:, 0:4, :])
                nc.scalar.dma_start(
                    ag_sb[1][:, 4:8, :],
                    ar_out[1].rearrange("r p c -> p r c")[:, 4:8, :])
                nc.vector.tensor_reduce(
                    gpk[1][:], ag_sb[1].rearrange("p r c -> p c r")[:],
                    AX.X, OP.add)

            with nc.named_scope("coeffs1"):
                bn_coeffs(1, hc[:, 1:2])

            # ---------- quantize1: S -> act1 codes ----------
            with nc.named_scope("quant1"):
                quant_phase(s_sb, act1, 1, "q1")

            # ---------- conv1 + residual epilogue ----------
            def epi1(co, r0, nr, ps):
                psf = ps.rearrange("p r c -> p (r c)")
                for (rl, n, img, h0) in _runs(r0, nr):
                    sl = psf[:, (rl - r0) * H:(rl - r0 + n) * H]
                    xt = x_fl[co][:, (img * H + h0) * H:(img * H + h0 + n) * H]
                    ot = runp.tile([P, n * H], F32, name=f"o_{co}_{rl}",
                                   tag="orun", bufs=5, padded_shape=[P, 9 * H])
                    nc.vector.scalar_tensor_tensor(
                        ot[:], sl, hc[:, 2:3], xt, OP.mult, OP.add)
                    nc.sync.dma_start(
                        ov[co * P:(co + 1) * P, img, h0:h0 + n, :],
                        ot.rearrange("p (a b) -> p a b", b=H)[:])

            with nc.named_scope("conv1"):
                conv(1, epi1)

    nc.compile()
    return nc


def _quantize_weight_host(w):
    """DoReFa 4-bit weight codes on host: returns ([P, 9, 2, 256] fp8 odd-int
    codes, scale M) with w_q = (M/15) * code."""
    t = np.tanh(np.asarray(w, np.float32))
    m = float(np.max(np.abs(t)))
    r = np.rint((7.5 / m) * t + 7.5)           # round(15 * (t/2M + 0.5)), RNE
    code = (2.0 * r - 15.0).astype(np.float32)  # odd ints in [-15, 15]
    # [o, i, kh, kw] -> [i, kh, kw, o] -> [ki, p, tap, o] -> [p, tap, ki, o]
    c2 = code.transpose(1, 2, 3, 0).reshape(2, P, 9, 256)
    wq = np.ascontiguousarray(c2.transpose(1, 2, 0, 3))
    return wq.astype(ml_dtypes.float8_e4m3), m


def _install_ntff_hook():
    """Provide antenv.axon_hooks (absent in this image) via ctypes so that
    run_bass_kernel_spmd(trace=True) can capture NTFF profiles."""
    try:
        from antenv.axon_hooks import get_axon_ntff_profile_hook  # noqa: F401
        return
    except ImportError:
        pass
    import contextlib
    import ctypes
    import types

    so_path = "/opt/axon/libaxon_pjrt.so"
    if not os.path.exists(so_path):
        return
    lib = ctypes.CDLL(so_path)
    if not hasattr(lib, "axon_start_nrt_profile"):
        return
    lib.axon_start_nrt_profile.argtypes = [ctypes.POINTER(ctypes.c_int64),
                                           ctypes.c_size_t]
    lib.axon_start_nrt_profile.restype = ctypes.c_int64
    lib.axon_stop_nrt_profile.argtypes = [ctypes.c_char_p]
    lib.axon_stop_nrt_profile.restype = ctypes.c_int64

    @contextlib.contextmanager
    def _hook(output_dir, device_ids):
        import jax
        jax.devices()
        if device_ids:
            ids = (ctypes.c_int64 * len(device_ids))(*device_ids)
            rc = lib.axon_start_nrt_profile(ids, len(device_ids))
        else:
            rc = lib.axon_start_nrt_profile(None, 0)
        if rc != 0:
            raise RuntimeError(f"axon_start_nrt_profile rc={rc}")
        try:
            yield
        finally:
            n = lib.axon_stop_nrt_profile(str(output_dir).encode())
            print(f"ntff profile: {n} file(s) written to {output_dir}")

    hook_holder = [_hook]
    mod = types.ModuleType("antenv.axon_hooks")
    mod.get_axon_ntff_profile_hook = lambda: hook_holder[0]
    mod.set_axon_ntff_profile_hook = lambda h: hook_holder.__setitem__(0, h)
    import antenv
    sys.modules["antenv.axon_hooks"] = mod
    antenv.axon_hooks = mod


_NC = None


def _get_nc():
    global _NC
    if _NC is None:
        _NC = build()
    return _NC


LAST_RESULTS = None


def kernel(x, bn0_gamma, bn0_beta, conv0_w, bn1_gamma, bn1_beta, conv1_w):
    global LAST_RESULTS
    nc = _get_nc()
    wq0, m0 = _quantize_weight_host(conv0_w)
    wq1, m1 = _quantize_weight_host(conv1_w)
    s0, s1 = m0 / 225.0, m1 / 225.0
    hconst = np.tile(np.array([[EPS, EPS / (s0 * s0), s1]], np.float32),
                     (P, 1))
    shared = {
        "wq0": wq0,
        "wq1": wq1,
        "hconst": hconst,
        "bn0_gamma": np.ascontiguousarray(bn0_gamma, np.float32),
        "bn0_beta": np.ascontiguousarray(bn0_beta, np.float32),
        "bn1_gamma": np.ascontiguousarray(bn1_gamma, np.float32),
        "bn1_beta": np.ascontiguousarray(bn1_beta, np.float32),
    }
    x = np.ascontiguousarray(x, np.float32)
    in_maps = [{"x": x[2 * c:2 * c + 2], **shared} for c in range(N_CORES)]
    trace = bool(int(os.environ.get("KERNEL_TRACE", "0")))
    if trace:
        _install_ntff_hook()
    res = bass_utils.run_bass_kernel_spmd(
        nc, in_maps, core_ids=list(range(N_CORES)), trace=trace)
    LAST_RESULTS = res
    return np.concatenate([res.results[c]["out"] for c in range(N_CORES)], axis=0)


# revision 32
# speedup vs baseline: 1.0284x; 1.0284x over previous
"""Trainium2 Bass kernel: PreActBlock with DoReFa 4-bit quantization (sync-BN).

  out = conv3x3(q(relu(BN1(conv3x3(q(relu(BN0(x))), qw(w0))))), qw(w1)) + x

Design (8 cores, data-parallel over batch 16 -> 2 images/core):
 - Quantized activations are integers 0..15 and quantized weights odd integers
   -15..15 (x scale).  Both are exact in fp8e4 (e4m3) and the PE accumulates
   in fp32, so every conv is computed EXACTLY as integer sums (|S| < 2^20).
 - Weight quantization (tanh / absmax / round to 4-bit codes) is data
   independent, so it runs on the HOST inside kernel(): the device loads
   1.2MB of ready fp8 codes instead of 4.7MB fp32 + on-chip tanh/max
   pipeline.  The two derived scalars the device needs (eps' = eps/(M0/225)^2
   for BN1-on-integer-stats, and M1/225 for the output descale) ride in a
   small replicated hconst tensor.
 - fp8 DoubleRow matmuls: contraction K=256 per instruction via the
   [P, 2, ...] interleaved layout (2x PE throughput).
 - BN batch stats are all-reduced across the 8 cores (sync-BN semantics).
 - Critical-path focus: gpsimd's queue carries ONLY stats work and the two
   AllReduce triggers (collectives must fire the moment their input DMA
   lands).  x is loaded in 8 chunks over the sync/scalar/vector/tensor
   queues; stats are computed per-chunk on three engines in parallel
   (scalar: img0 sum-of-squares, gpsimd: img0 sum, vector: img1 bn_stats).
 - BN coefficient chains are short: EPS folds into the Sqrt activation bias,
   rsqrt = vector.reciprocal + scalar Sqrt (no Newton; tolerance budget is
   ~2e-2 and quantization rounding dominates), and the BN1 chain runs
   directly on integer stats (scale folded into eps').
 - Act layout [P, row, ki, 64]: ki innermost (stride 64 so DoubleRow APs are
   legal) keeps sub-tile dependency intervals row-tight, so conv matmuls can
   start as soon as the first quantized rows land.
 - x and the intermediate conv0 output S live entirely in SBUF (no DRAM
   spill/reload), so quant1 -> conv1 starts right after the second AllReduce.
"""
import os
import sys

sys.path.insert(0, "/opt/trn_rl_repo")

import ml_dtypes
import numpy as np

import concourse.bacc as bacc
import concourse.mybir as mybir
from concourse import tile
from concourse import bass_utils

F32 = mybir.dt.float32
FP8 = mybir.dt.float8e4
I8 = mybir.dt.int8
AX = mybir.AxisListType
OP = mybir.AluOpType
AF = mybir.ActivationFunctionType
PM = mybir.MatmulPerfMode

P = 128
N_CORES = 8
IMG = 2              # images per core
H = 56
ROWS = 116           # 2 images x (1 pad + 56 + 1 pad) rows
CW = 64              # padded column stride of act rows (ki stride, %16 == 0)
CNT = 50176.0        # global BN count: 16 * 56 * 56
EPS = 1e-5

# per-image 9-row output windows (junk boundary rows 57/58 never computed)
WINDOWS = ([(1 + 9 * k, 9) for k in range(6)] + [(55, 2)] +
           [(59 + 9 * k, 9) for k in range(6)] + [(113, 2)])
GROUPS = [WINDOWS[0:2], WINDOWS[2:6], WINDOWS[6:10], WINDOWS[10:14]]
# tap order: full-width tap (dy=0,dx=1) first so start=True covers all columns
TAPS = [(0, 1), (0, 0), (0, 2), (1, 0), (1, 1), (1, 2), (2, 0), (2, 1), (2, 2)]

# x chunk order: img1 early (vector's bn_stats tail is longest), interleaved
# with img0 (scalar + gpsimd).  Only gpsimd/sync/scalar queues can trigger
# DMAs; gpsimd fires the two first-to-process chunks at the very top of its
# queue (its stats ops follow them in program order).
X_ORDER = [(1, 0, 0), (0, 0, 0), (1, 0, 1), (0, 0, 1),
           (1, 1, 0), (0, 1, 0), (1, 1, 1), (0, 1, 1)]


def _runs(r0, nr):
    """Interior row-runs of a window: (logical_row, nrows, img, h0)."""
    out = []
    for lo, hi, img, base in ((1, 56, 0, 1), (59, 114, 1, 59)):
        a, b = max(r0, lo), min(r0 + nr - 1, hi)
        if a <= b:
            out.append((a, b - a + 1, img, a - base))
    return out


def build():
    nc = bacc.Bacc("TRN2", target_bir_lowering=False, debug=False,
                   enable_asserts=False, num_devices=N_CORES)

    x_d = nc.dram_tensor("x", [IMG, 256, H, H], F32, kind="ExternalInput")
    # host-quantized weight codes, [ci_lo, tap, ki, co] fp8 (odd ints)
    wq_d = [nc.dram_tensor("wq0", [P, 9, 2, 256], FP8, kind="ExternalInput"),
            nc.dram_tensor("wq1", [P, 9, 2, 256], FP8, kind="ExternalInput")]
    g_d = [nc.dram_tensor("bn0_gamma", [256], F32, kind="ExternalInput"),
           nc.dram_tensor("bn1_gamma", [256], F32, kind="ExternalInput")]
    b_d = [nc.dram_tensor("bn0_beta", [256], F32, kind="ExternalInput"),
           nc.dram_tensor("bn1_beta", [256], F32, kind="ExternalInput")]
    # replicated host constants:
    #   col0 = EPS, col1 = eps' = EPS/(M0/225)^2, col2 = M1/225
    hc_d = nc.dram_tensor("hconst", [P, 3], F32, kind="ExternalInput")
    out_d = nc.dram_tensor("out", [IMG, 256, H, H], F32, kind="ExternalOutput")

    xv = x_d.ap().rearrange("n c h w -> c n h w")       # [256, 2, 56, 56]
    ov = out_d.ap().rearrange("n c h w -> c n h w")

    with tile.TileContext(nc) as tc:
        with tc.tile_pool(name="act", bufs=1) as actp, \
             tc.tile_pool(name="wtp", bufs=1) as wtp, \
             tc.tile_pool(name="qt", bufs=3) as qtp, \
             tc.tile_pool(name="run", bufs=6) as runp, \
             tc.tile_pool(name="st", bufs=1) as stp, \
             tc.tile_pool(name="ps", bufs=8, space="PSUM") as psp, \
             tc.tile_pool(name="dram", bufs=1, space="DRAM") as drp:

            # ---------- static tiles ----------
            # act layout: [P, row, ki, CW]; image cols 0..55, cols 56..63 are
            # never-read filler that pads the ki stride to 64.
            act0 = actp.tile([P, ROWS, 2, CW], FP8, name="act0")
            act1 = actp.tile([P, ROWS, 2, CW], FP8, name="act1")
            actv = [act0.rearrange("p r k c -> p k r c"),
                    act1.rearrange("p r k c -> p k r c")]
            # quantized weight codes, [ci_lo, tap, ki, co] fp8
            wT = [wtp.tile([P, 9, 2, 256], FP8, name=f"w{v}T") for v in range(2)]
            # full x and conv0-integer-output S, SBUF-resident per co half
            x_sb = [actp.tile([P, IMG, H, H], F32, name=f"x_sb_{c}")
                    for c in range(2)]
            s_sb = [actp.tile([P, IMG, H, H], F32, name=f"s_sb_{c}")
                    for c in range(2)]
            x_fl = [t.rearrange("p i h w -> p (i h w)") for t in x_sb]
            s_fl = [t.rearrange("p i h w -> p (i h w)") for t in s_sb]
            ar_in = [drp.tile([P, 4], F32, name=f"ar_in_{i}") for i in range(2)]
            # AllGather output: rank-major blocks [r][p][c] in DRAM
            ar_out = [drp.tile([N_CORES, P, 4], F32, name=f"ar_out_{i}")
                      for i in range(2)]
            ag_sb = [stp.tile([P, 8, 4], F32, name=f"ag_sb_{i}")
                     for i in range(2)]

            # stats / small vectors
            xbn = [stp.tile([P, 8, 6], F32, name=f"xbn_{c}") for c in range(2)]
            gsx = [stp.tile([P, 2], F32, name=f"gsx_{c}") for c in range(2)]
            sqx = [stp.tile([P, 2], F32, name=f"sqx_{c}") for c in range(2)]
            sumS = [stp.tile([P, 16], F32, name=f"sumS_{c}") for c in range(2)]
            sqS = [stp.tile([P, 16], F32, name=f"sqS_{c}") for c in range(2)]
            gvec = [stp.tile([P, 2], F32, name=f"g{v}") for v in range(2)]
            bvec = [stp.tile([P, 2], F32, name=f"b{v}") for v in range(2)]
            avec = [stp.tile([P, 2], F32, name=f"a{v}") for v in range(2)]
            bbvec = [stp.tile([P, 2], F32, name=f"bb{v}") for v in range(2)]
            hc = stp.tile([P, 3], F32, name="hc")
            pk = [stp.tile([P, 4], F32, name=f"pk_{i}") for i in range(2)]
            gpk = [stp.tile([P, 4], F32, name=f"gpk_{i}") for i in range(2)]

            def vtile(name, w=1):
                return stp.tile([P, w], F32, name=name, tag="vtmp", bufs=8,
                                padded_shape=[P, 4])

            # ---------- x load: paced triggers on the scalar queue ----------
            # The DMA engines fair-share bandwidth over all pending
            # transfers, so triggering everything up front makes every chunk
            # land at the END of the load.  Instead the scalar queue
            # interleaves triggers with its own per-chunk stats passes, so
            # chunks land in processing order and stats pipeline with the
            # load.  img0 sum-of-squares on scalar; img0 sum + img1 bn_stats
            # on vector.
            def xtrig(img, blk, c):
                h0 = blk * 28
                nc.scalar.dma_start(x_sb[c][:, img, h0:h0 + 28, :],
                                    xv[c * P:(c + 1) * P, img, h0:h0 + 28, :])

            def sq_pass(blk, c):
                h0 = blk * 28
                nc.scalar.activation(
                    s_sb[c][:, 0, h0:h0 + 28, :],
                    x_sb[c][:, 0, h0:h0 + 28, :], AF.Square,
                    accum_out=sqx[c][:, blk:blk + 1])

            def sum_pass(blk, c):
                h0 = blk * 28
                nc.vector.tensor_scalar(
                    s_sb[c][:, 1, h0:h0 + 28, :],
                    x_sb[c][:, 0, h0:h0 + 28, :], 1.0, 0.0,
                    OP.mult, OP.add, accum_out=gsx[c][:, blk:blk + 1])

            def bn_pass(blk, c):
                h0 = blk * 28
                fl = x_fl[c][:, (H + h0) * H:(H + h0 + 28) * H]
                k = blk * 4
                for cc in range(4):
                    nc.vector.bn_stats(xbn[c][:, k + cc, :],
                                       fl[:, cc * 392:(cc + 1) * 392])

            with nc.named_scope("stats0"):
                xtrig(1, 0, 0)
                xtrig(0, 0, 0)
                xtrig(1, 0, 1)
                xtrig(0, 0, 1)
                sq_pass(0, 0)          # scalar queue: waits chunk (0,0,0)
                xtrig(1, 1, 0)
                xtrig(0, 1, 0)
                sq_pass(0, 1)
                xtrig(0, 1, 1)         # img0/c1 lands 7th: its Square pass
                xtrig(1, 1, 1)         # overlaps the final img1 bn_stats
                sq_pass(1, 0)
                sq_pass(1, 1)
                # pre-touch Sqrt while scalar is idle: loads its activation
                # table now instead of on the post-AllGather critical path
                # (the ag-back half-DMA below blocks the scalar queue until
                # the collective completes)
                twarm = vtile("sqrt_warm")
                nc.scalar.activation(twarm[:], sqx[1][:, 1:2], AF.Sqrt)
                # vector consumes in landing order
                bn_pass(0, 0)
                sum_pass(0, 0)
                bn_pass(0, 1)
                sum_pass(0, 1)
                bn_pass(1, 0)
                sum_pass(1, 0)
                sum_pass(1, 1)
                bn_pass(1, 1)

            # weight-code + param loads: scalar queue, after the x pacers
            for v in range(2):
                nc.scalar.dma_start(wT[v][:], wq_d[v].ap()[:])
            for v in range(2):
                for c in range(2):
                    nc.scalar.dma_start(gvec[v][:, c:c + 1],
                                        g_d[v].ap()[c * P:(c + 1) * P])
                    nc.scalar.dma_start(bvec[v][:, c:c + 1],
                                        b_d[v].ap()[c * P:(c + 1) * P])
            nc.scalar.dma_start(hc[:], hc_d.ap()[:])

            # ---------- act pad-row zeroing (vector, overlaps x DMA) --------
            with nc.named_scope("memset"):
                for t in (act0, act1):
                    for r in (0, 57, 58, 115):
                        nc.vector.memset(t[:, r, :, 0:H], 0.0)

            with nc.named_scope("stats0c"):
                for c in range(2):
                    mv = stp.tile([P, 2], F32, name=f"mv0_{c}")
                    nc.vector.bn_aggr(mv[:], xbn[c][:, 0:8, :])
                    sv_ = vtile(f"sv_{c}")
                    nc.vector.scalar_tensor_tensor(
                        sv_[:], mv[:, 0:1], 3136.0, gsx[c][:, 0:1],
                        OP.mult, OP.add)
                    nc.vector.tensor_add(pk[0][:, c:c + 1], sv_[:],
                                         gsx[c][:, 1:2])
                    m2 = vtile(f"xm2_{c}")
                    nc.vector.tensor_mul(m2[:], mv[:, 0:1], mv[:, 0:1])
                    vp = vtile(f"xvp_{c}")
                    nc.vector.tensor_add(vp[:], mv[:, 1:2], m2[:])
                    qv_ = vtile(f"qv_{c}")
                    nc.vector.scalar_tensor_tensor(
                        qv_[:], vp[:], 3136.0, sqx[c][:, 0:1],
                        OP.mult, OP.add)
                    nc.vector.tensor_add(pk[0][:, 2 + c:3 + c], qv_[:],
                                         sqx[c][:, 1:2])
                nc.gpsimd.dma_start(ar_in[0][:], pk[0][:])
                nc.gpsimd.collective_compute(
                    "AllGather", OP.bypass,
                    replica_groups=[list(range(N_CORES))],
                    ins=[ar_in[0].opt()], outs=[ar_out[0].opt()])
                # gather-back split across two queues: the [P,8,4] view is
                # 16B-per-descriptor strided, so halving it in parallel
                # roughly halves the flight time
                nc.sync.dma_start(
                    ag_sb[0][:, 0:4, :],
                    ar_out[0].rearrange("r p c -> p r c")[:, 0:4, :])
                nc.scalar.dma_start(
                    ag_sb[0][:, 4:8, :],
                    ar_out[0].rearrange("r p c -> p r c")[:, 4:8, :])
                nc.vector.tensor_reduce(
                    gpk[0][:], ag_sb[0].rearrange("p r c -> p c r")[:],
                    AX.X, OP.add)

            # ---------- BN coefficients: z = a*S + b on [P, 2] ----------
            # a = 15*gamma*rsqrt(var + eps), b = 15*beta - mean*a.
            # For v=1 the stats are raw integer-S stats and eps_bias = eps'
            # (scale folded in on the host), so the chain is identical.
            def bn_coeffs(v, eps_bias):
                me = vtile(f"me{v}", 4)
                nc.vector.tensor_scalar(me[:], gpk[v][:, 0:4], 1.0 / CNT,
                                        None, OP.mult)
                mean, ex2 = me[:, 0:2], me[:, 2:4]
                m2 = vtile(f"m2{v}", 2)
                nc.vector.tensor_mul(m2[:], mean, mean)
                vpe = vtile(f"vp{v}", 2)
                nc.vector.tensor_sub(vpe[:], ex2, m2[:])
                y = vtile(f"y{v}", 2)
                nc.scalar.activation(y[:], vpe[:], AF.Sqrt, bias=eps_bias)
                rr = vtile(f"rr{v}", 2)
                nc.vector.reciprocal(rr[:], y[:])
                grs = vtile(f"gr{v}", 2)
                nc.vector.tensor_mul(grs[:], gvec[v][:], rr[:])
                nc.vector.tensor_scalar(avec[v][:], grs[:], 15.0, None,
                                        OP.mult)
                mg = vtile(f"mg{v}", 2)
                nc.vector.tensor_mul(mg[:], mean, avec[v][:])
                nc.vector.scalar_tensor_tensor(
                    bbvec[v][:], bvec[v][:], 15.0, mg[:],
                    OP.mult, OP.subtract)

            with nc.named_scope("coeffs0"):
                bn_coeffs(0, hc[:, 0:1])

            # ---------- quantize pipeline ----------
            # stage1: int8(relu(a*x+b)) on scalar (RNE cast rounds)
            # stage2: vector min(.,15) -> fp8 act codes
            def quantize_block(src_t, act_t, c, img, h0, nr, v, names):
                lr = img * 58 + 1 + h0   # logical row
                src_ch = src_t[:, img, h0:h0 + nr, :]
                u = qtp.tile([P, nr, H], I8, name=names + "u", tag="qu",
                             bufs=4, padded_shape=[P, 28, H])
                nc.scalar.activation(u[:], src_ch, AF.Relu,
                                     bias=bbvec[v][:, c:c + 1],
                                     scale=avec[v][:, c:c + 1])
                nc.vector.tensor_scalar(act_t[:, lr:lr + nr, c, 0:H], u[:],
                                        15.0, None, OP.min)

            def quant_phase(src, act_t, v, tag):
                # first block = exactly the 11 input rows of conv window 1,
                # so the first matmul fires as soon as possible after the
                # AllReduce
                for c in range(2):
                    quantize_block(src[c], act_t, c, 0, 0, 11, v,
                                   f"{tag}_{c}00a")
                for c in range(2):
                    quantize_block(src[c], act_t, c, 0, 11, 17, v,
                                   f"{tag}_{c}00b")
                for img in range(IMG):
                    for blk in range(2):
                        if img == 0 and blk == 0:
                            continue
                        for c in range(2):
                            quantize_block(src[c], act_t, c, img, blk * 28,
                                           28, v, f"{tag}_{c}{img}{blk}")

            with nc.named_scope("quant0"):
                quant_phase(x_sb, act0, 0, "q0")

            # ---------- conv (shared), fp8 DoubleRow, K=256 per matmul -------
            def conv(v, epilogue):
                av = actv[v]
                for gi, grp in enumerate(GROUPS):
                    for co in range(2):
                        psums = []
                        for wi, (r0, nr) in enumerate(grp):
                            ps = psp.tile([P, nr, H], F32,
                                          name=f"ps{v}_{gi}_{co}_{wi}",
                                          tag="psw", padded_shape=[P, 9, H])
                            psums.append(ps)
                        for ti, (dy, dx) in enumerate(TAPS):
                            tap = dy * 3 + dx
                            wlo, whi = max(0, 1 - dx), min(H, H + 1 - dx)
                            jlo = max(0, dx - 1)
                            lhsT = wT[v][:, tap, :, co * P:(co + 1) * P]
                            first = ti == 0
                            last = ti == 8
                            for wi, (r0, nr) in enumerate(grp):
                                rows = slice(r0 + dy - 1, r0 + dy - 1 + nr)
                                rhs = av[:, :, rows, jlo:jlo + whi - wlo]
                                if dx == 1:
                                    out = psums[wi][:, :, :]
                                else:
                                    out = psums[wi][:, :, wlo:whi]
                                nc.tensor.matmul(out, lhsT, rhs,
                                                 start=first, stop=last,
                                                 perf_mode=PM.DoubleRow)
                        for wi, (r0, nr) in enumerate(grp):
                            epilogue(co, r0, nr, psums[wi])

            # ---------- conv0 epilogue: S -> SBUF + interior sums ----------
            # sum on scalar (psum drain w/ accum), square-sum on gpsimd
            slot_idx = [0, 0]

            def epi0(co, r0, nr, ps):
                psf = ps.rearrange("p r c -> p (r c)")
                for (rl, n, img, h0) in _runs(r0, nr):
                    sl = psf[:, (rl - r0) * H:(rl - r0 + n) * H]
                    dst = s_fl[co][:, (img * H + h0) * H:(img * H + h0 + n) * H]
                    k = slot_idx[co]
                    slot_idx[co] += 1
                    nc.scalar.activation(dst, sl, AF.Identity,
                                         accum_out=sumS[co][:, k:k + 1])
                    sq = runp.tile([P, n * H], F32, name=f"sq_{co}_{rl}",
                                   tag="sq", bufs=2, padded_shape=[P, 9 * H])
                    nc.vector.scalar_tensor_tensor(
                        sq[:], dst, 0.0, dst, OP.bypass, OP.mult,
                        accum_out=sqS[co][:, k:k + 1])

            with nc.named_scope("conv0"):
                conv(0, epi0)

            # ---------- BN1 stats + AR ----------
            with nc.named_scope("stats1"):
                for c in range(2):
                    ns = slot_idx[c]
                    nc.vector.tensor_reduce(pk[1][:, c:c + 1],
                                            sumS[c][:, 0:ns], AX.X, OP.add)
                    nc.vector.tensor_reduce(pk[1][:, 2 + c:3 + c],
                                            sqS[c][:, 0:ns], AX.X, OP.add)
                nc.gpsimd.dma_start(ar_in[1][:], pk[1][:])
                nc.gpsimd.collective_compute(
                    "AllGather", OP.bypass,
                    replica_groups=[list(range(N_CORES))],
                    ins=[ar_in[1].opt()], outs=[ar_out[1].opt()])
                # gather-back split across two queues: the [P,8,4] view is
                # 16B-per-descriptor strided, so halving it in parallel
                # roughly halves the flight time
                nc.sync.dma_start(
                    ag_sb[1][:, 0:4, :],
                    ar_out[1].rearrange("r p c -> p r c")[:, 0:4, :])
                nc.scalar.dma_start(
                    ag_sb[1][:, 4:8, :],
                    ar_out[1].rearrange("r p c -> p r c")[:, 4:8, :])
                nc.vector.tensor_reduce(
                    gpk[1][:], ag_sb[1].rearrange("p r c -> p c r")[:],
                    AX.X, OP.add)

            with nc.named_scope("coeffs1"):
                bn_coeffs(1, hc[:, 1:2])

            # ---------- quantize1: S -> act1 codes ----------
            with nc.named_scope("quant1"):
                quant_phase(s_sb, act1, 1, "q1")

            # ---------- conv1 + residual epilogue ----------
            def epi1(co, r0, nr, ps):
                psf = ps.rearrange("p r c -> p (r c)")
                for (rl, n, img, h0) in _runs(r0, nr):
                    sl = psf[:, (rl - r0) * H:(rl - r0 + n) * H]
                    xt = x_fl[co][:, (img * H + h0) * H:(img * H + h0 + n) * H]
                    ot = runp.tile([P, n * H], F32, name=f"o_{co}_{rl}",
                                   tag="orun", bufs=5, padded_shape=[P, 9 * H])
                    nc.vector.scalar_tensor_tensor(
                        ot[:], sl, hc[:, 2:3], xt, OP.mult, OP.add)
                    nc.sync.dma_start(
                        ov[co * P:(co + 1) * P, img, h0:h0 + n, :],
                        ot.rearrange("p (a b) -> p a b", b=H)[:])

            with nc.named_scope("conv1"):
                conv(1, epi1)

    nc.compile()
    return nc


def _quantize_weight_host(w):
    """DoReFa 4-bit weight codes on host: returns ([P, 9, 2, 256] fp8 odd-int
    codes, scale M) with w_q = (M/15) * code."""
    t = np.tanh(np.asarray(w, np.float32))
    m = float(np.max(np.abs(t)))
    r = np.rint((7.5 / m) * t + 7.5)           # round(15 * (t/2M + 0.5)), RNE
    code = (2.0 * r - 15.0).astype(np.float32)  # odd ints in [-15, 15]
    # [o, i, kh, kw] -> [i, kh, kw, o] -> [ki, p, tap, o] -> [p, tap, ki, o]
    c2 = code.transpose(1, 2, 3, 0).reshape(2, P, 9, 256)
    wq = np.ascontiguousarray(c2.transpose(1, 2, 0, 3))
    return wq.astype(ml_dtypes.float8_e4m3), m


def _install_ntff_hook():
    """Provide antenv.axon_hooks (absent in this image) via ctypes so that
    run_bass_kernel_spmd(trace=True) can capture NTFF profiles."""
    try:
        from antenv.axon_hooks import get_axon_ntff_profile_hook  # noqa: F401
        return
    except ImportError:
        pass
    import contextlib
    import ctypes
    import types

    so_path = "/opt/axon/libaxon_pjrt.so"
    if not os.path.exists(so_path):
        return
    lib = ctypes.CDLL(so_path)
    if not hasattr(lib, "axon_start_nrt_profile"):
        return
    lib.axon_start_nrt_profile.argtypes = [ctypes.POINTER(ctypes.c_int64),
                                           ctypes.c_size_t]
    lib.axon_start_nrt_profile.restype = ctypes.c_int64
    lib.axon_stop_nrt_profile.argtypes = [ctypes.c_char_p]
    lib.axon_stop_nrt_profile.restype = ctypes.c_int64

    @contextlib.contextmanager
    def _hook(output_dir, device_ids):
        import jax
        jax.devices()
        if device_ids:
            ids = (ctypes.c_int64 * len(device_ids))(*device_ids)
            rc = lib.axon_start_nrt_profile(ids, len(device_ids))
        else:
            rc = lib.axon_start_nrt_profile(None, 0)
        if rc != 0:
            raise RuntimeError(f"axon_start_nrt_profile rc={rc}")
        try:
            yield
        finally:
            n = lib.axon_stop_nrt_profile(str(output_dir).encode())
            print(f"ntff profile: {n} file(s) written to {output_dir}")

    hook_holder = [_hook]
    mod = types.ModuleType("antenv.axon_hooks")
    mod.get_axon_ntff_profile_hook = lambda: hook_holder[0]
    mod.set_axon_ntff_profile_hook = lambda h: hook_holder.__setitem__(0, h)
    import antenv
    sys.modules["antenv.axon_hooks"] = mod
    antenv.axon_hooks = mod


_NC = None


def _get_nc():
    global _NC
    if _NC is None:
        _NC = build()
    return _NC


LAST_RESULTS = None


def kernel(x, bn0_gamma, bn0_beta, conv0_w, bn1_gamma, bn1_beta, conv1_w):
    global LAST_RESULTS
    nc = _get_nc()
    wq0, m0 = _quantize_weight_host(conv0_w)
    wq1, m1 = _quantize_weight_host(conv1_w)
    s0, s1 = m0 / 225.0, m1 / 225.0
    hconst = np.tile(np.array([[EPS, EPS / (s0 * s0), s1]], np.float32),
                     (P, 1))
    shared = {
        "wq0": wq0,
        "wq1": wq1,
        "hconst": hconst,
        "bn0_gamma": np.ascontiguousarray(bn0_gamma, np.float32),
        "bn0_beta": np.ascontiguousarray(bn0_beta, np.float32),
        "bn1_gamma": np.ascontiguousarray(bn1_gamma, np.float32),
        "bn1_beta": np.ascontiguousarray(bn1_beta, np.float32),
    }
    x = np.ascontiguousarray(x, np.float32)
    in_maps = [{"x": x[2 * c:2 * c + 2], **shared} for c in range(N_CORES)]
    trace = bool(int(os.environ.get("KERNEL_TRACE", "0")))
    if trace:
        _install_ntff_hook()
    res = bass_utils.run_bass_kernel_spmd(
        nc, in_maps, core_ids=list(range(N_CORES)), trace=trace)
    LAST_RESULTS = res
    return np.concatenate([res.results[c]["out"] for c in range(N_CORES)], axis=0)


# revision 34
# speedup vs baseline: 1.0744x; 1.0447x over previous
"""Trainium2 Bass kernel: PreActBlock with DoReFa 4-bit quantization (sync-BN).

  out = conv3x3(q(relu(BN1(conv3x3(q(relu(BN0(x))), qw(w0))))), qw(w1)) + x

Design (8 cores, data-parallel over batch 16 -> 2 images/core):
 - Quantized activations are integers 0..15 and quantized weights odd integers
   -15..15 (x scale).  Both are exact in fp8e4 (e4m3) and the PE accumulates
   in fp32, so every conv is computed EXACTLY as integer sums (|S| < 2^20).
 - Weight quantization (tanh / absmax / round to 4-bit codes) is data
   independent, so it runs on the HOST inside kernel(): the device loads
   1.2MB of ready fp8 codes instead of 4.7MB fp32 + on-chip tanh/max
   pipeline.  The two derived scalars the device needs (eps' = eps/(M0/225)^2
   for BN1-on-integer-stats, and M1/225 for the output descale) ride in a
   small replicated hconst tensor.
 - fp8 DoubleRow matmuls: contraction K=256 per instruction via the
   [P, 2, ...] interleaved layout (2x PE throughput).
 - BN batch stats are all-reduced across the 8 cores (sync-BN semantics).
 - Critical-path focus: gpsimd's queue carries ONLY stats work and the two
   AllReduce triggers (collectives must fire the moment their input DMA
   lands).  x is loaded in 8 chunks over the sync/scalar/vector/tensor
   queues; stats are computed per-chunk on three engines in parallel
   (scalar: img0 sum-of-squares, gpsimd: img0 sum, vector: img1 bn_stats).
 - BN coefficient chains are short: EPS folds into the Sqrt activation bias,
   rsqrt = vector.reciprocal + scalar Sqrt (no Newton; tolerance budget is
   ~2e-2 and quantization rounding dominates), and the BN1 chain runs
   directly on integer stats (scale folded into eps').
 - Act layout [P, row, ki, 64]: ki innermost (stride 64 so DoubleRow APs are
   legal) keeps sub-tile dependency intervals row-tight, so conv matmuls can
   start as soon as the first quantized rows land.
 - x and the intermediate conv0 output S live entirely in SBUF (no DRAM
   spill/reload), so quant1 -> conv1 starts right after the second AllReduce.
"""
import os
import sys

sys.path.insert(0, "/opt/trn_rl_repo")

import ml_dtypes
import numpy as np

import concourse.bacc as bacc
import concourse.mybir as mybir
from concourse import tile
from concourse import bass_utils

F32 = mybir.dt.float32
FP8 = mybir.dt.float8e4
I8 = mybir.dt.int8
AX = mybir.AxisListType
OP = mybir.AluOpType
AF = mybir.ActivationFunctionType
PM = mybir.MatmulPerfMode

P = 128
N_CORES = 8
IMG = 2              # images per core
H = 56
ROWS = 116           # 2 images x (1 pad + 56 + 1 pad) rows
CW = 64              # padded column stride of act rows (ki stride, %16 == 0)
CNT = 50176.0        # global BN count: 16 * 56 * 56
EPS = 1e-5

# per-image 9-row output windows (junk boundary rows 57/58 never computed)
WINDOWS = ([(1 + 9 * k, 9) for k in range(6)] + [(55, 2)] +
           [(59 + 9 * k, 9) for k in range(6)] + [(113, 2)])
GROUPS = [WINDOWS[0:2], WINDOWS[2:6], WINDOWS[6:10], WINDOWS[10:14]]
# tap order: full-width tap (dy=0,dx=1) first so start=True covers all columns
TAPS = [(0, 1), (0, 0), (0, 2), (1, 0), (1, 1), (1, 2), (2, 0), (2, 1), (2, 2)]

# x chunk order: img1 early (vector's bn_stats tail is longest), interleaved
# with img0 (scalar + gpsimd).  Only gpsimd/sync/scalar queues can trigger
# DMAs; gpsimd fires the two first-to-process chunks at the very top of its
# queue (its stats ops follow them in program order).
X_ORDER = [(1, 0, 0), (0, 0, 0), (1, 0, 1), (0, 0, 1),
           (1, 1, 0), (0, 1, 0), (1, 1, 1), (0, 1, 1)]


def _runs(r0, nr):
    """Interior row-runs of a window: (logical_row, nrows, img, h0)."""
    out = []
    for lo, hi, img, base in ((1, 56, 0, 1), (59, 114, 1, 59)):
        a, b = max(r0, lo), min(r0 + nr - 1, hi)
        if a <= b:
            out.append((a, b - a + 1, img, a - base))
    return out


def build():
    nc = bacc.Bacc("TRN2", target_bir_lowering=False, debug=False,
                   enable_asserts=False, num_devices=N_CORES)

    x_d = nc.dram_tensor("x", [IMG, 256, H, H], F32, kind="ExternalInput")
    # host-quantized weight codes, [ci_lo, tap, ki, co] fp8 (odd ints)
    wq_d = [nc.dram_tensor("wq0", [P, 9, 2, 256], FP8, kind="ExternalInput"),
            nc.dram_tensor("wq1", [P, 9, 2, 256], FP8, kind="ExternalInput")]
    g_d = [nc.dram_tensor("bn0_gamma", [256], F32, kind="ExternalInput"),
           nc.dram_tensor("bn1_gamma", [256], F32, kind="ExternalInput")]
    b_d = [nc.dram_tensor("bn0_beta", [256], F32, kind="ExternalInput"),
           nc.dram_tensor("bn1_beta", [256], F32, kind="ExternalInput")]
    # replicated host constants:
    #   col0 = EPS, col1 = eps' = EPS/(M0/225)^2, col2 = M1/225
    hc_d = nc.dram_tensor("hconst", [P, 3], F32, kind="ExternalInput")
    out_d = nc.dram_tensor("out", [IMG, 256, H, H], F32, kind="ExternalOutput")

    xv = x_d.ap().rearrange("n c h w -> c n h w")       # [256, 2, 56, 56]
    ov = out_d.ap().rearrange("n c h w -> c n h w")

    with tile.TileContext(nc) as tc:
        with tc.tile_pool(name="act", bufs=1) as actp, \
             tc.tile_pool(name="wtp", bufs=1) as wtp, \
             tc.tile_pool(name="qt", bufs=3) as qtp, \
             tc.tile_pool(name="run", bufs=6) as runp, \
             tc.tile_pool(name="st", bufs=1) as stp, \
             tc.tile_pool(name="ps", bufs=8, space="PSUM") as psp, \
             tc.tile_pool(name="dram", bufs=1, space="DRAM") as drp:

            # ---------- static tiles ----------
            # act layout: [P, row, ki, CW]; image cols 0..55, cols 56..63 are
            # never-read filler that pads the ki stride to 64.
            act0 = actp.tile([P, ROWS, 2, CW], FP8, name="act0")
            act1 = actp.tile([P, ROWS, 2, CW], FP8, name="act1")
            actv = [act0.rearrange("p r k c -> p k r c"),
                    act1.rearrange("p r k c -> p k r c")]
            # quantized weight codes, [ci_lo, tap, ki, co] fp8
            wT = [wtp.tile([P, 9, 2, 256], FP8, name=f"w{v}T") for v in range(2)]
            # full x and conv0-integer-output S, SBUF-resident per co half
            x_sb = [actp.tile([P, IMG, H, H], F32, name=f"x_sb_{c}")
                    for c in range(2)]
            s_sb = [actp.tile([P, IMG, H, H], F32, name=f"s_sb_{c}")
                    for c in range(2)]
            x_fl = [t.rearrange("p i h w -> p (i h w)") for t in x_sb]
            s_fl = [t.rearrange("p i h w -> p (i h w)") for t in s_sb]
            ar_in = [drp.tile([P, 4], F32, name=f"ar_in_{i}") for i in range(2)]
            # AllGather output: rank-major blocks [r][p][c] in DRAM
            ar_out = [drp.tile([N_CORES, P, 4], F32, name=f"ar_out_{i}")
                      for i in range(2)]
            ag_sb = [stp.tile([P, 8, 4], F32, name=f"ag_sb_{i}")
                     for i in range(2)]

            # stats / small vectors
            xbn = [stp.tile([P, 8, 6], F32, name=f"xbn_{c}") for c in range(2)]
            gsx = [stp.tile([P, 2], F32, name=f"gsx_{c}") for c in range(2)]
            sqx = [stp.tile([P, 2], F32, name=f"sqx_{c}") for c in range(2)]
            sumS = [stp.tile([P, 16], F32, name=f"sumS_{c}") for c in range(2)]
            sqS = [stp.tile([P, 16], F32, name=f"sqS_{c}") for c in range(2)]
            gvec = [stp.tile([P, 2], F32, name=f"g{v}") for v in range(2)]
            bvec = [stp.tile([P, 2], F32, name=f"b{v}") for v in range(2)]
            avec = [stp.tile([P, 2], F32, name=f"a{v}") for v in range(2)]
            bbvec = [stp.tile([P, 2], F32, name=f"bb{v}") for v in range(2)]
            hc = stp.tile([P, 3], F32, name="hc")
            pk = [stp.tile([P, 4], F32, name=f"pk_{i}") for i in range(2)]
            gpk = [stp.tile([P, 4], F32, name=f"gpk_{i}") for i in range(2)]

            def vtile(name, w=1):
                return stp.tile([P, w], F32, name=name, tag="vtmp", bufs=8,
                                padded_shape=[P, 4])

            # ---------- x load: paced triggers on the scalar queue ----------
            # The DMA engines fair-share bandwidth over all pending
            # transfers, so triggering everything up front makes every chunk
            # land at the END of the load.  Instead the scalar queue
            # interleaves triggers with its own per-chunk stats passes, so
            # chunks land in processing order and stats pipeline with the
            # load.  img0 sum-of-squares on scalar; img0 sum + img1 bn_stats
            # on vector.
            def xtrig(img, blk, c):
                h0 = blk * 28
                nc.scalar.dma_start(x_sb[c][:, img, h0:h0 + 28, :],
                                    xv[c * P:(c + 1) * P, img, h0:h0 + 28, :])

            def sq_pass(blk, c):
                h0 = blk * 28
                nc.scalar.activation(
                    s_sb[c][:, 0, h0:h0 + 28, :],
                    x_sb[c][:, 0, h0:h0 + 28, :], AF.Square,
                    accum_out=sqx[c][:, blk:blk + 1])

            def sum_pass(blk, c):
                h0 = blk * 28
                nc.vector.tensor_scalar(
                    s_sb[c][:, 1, h0:h0 + 28, :],
                    x_sb[c][:, 0, h0:h0 + 28, :], 1.0, 0.0,
                    OP.mult, OP.add, accum_out=gsx[c][:, blk:blk + 1])

            def bn_pass(blk, c):
                h0 = blk * 28
                fl = x_fl[c][:, (H + h0) * H:(H + h0 + 28) * H]
                k = blk * 4
                for cc in range(4):
                    nc.vector.bn_stats(xbn[c][:, k + cc, :],
                                       fl[:, cc * 392:(cc + 1) * 392])

            with nc.named_scope("stats0"):
                xtrig(1, 0, 0)
                xtrig(0, 0, 0)
                xtrig(1, 0, 1)
                xtrig(0, 0, 1)
                sq_pass(0, 0)          # scalar queue: waits chunk (0,0,0)
                xtrig(1, 1, 0)
                xtrig(0, 1, 0)
                sq_pass(0, 1)
                xtrig(1, 1, 1)
                xtrig(0, 1, 1)
                sq_pass(1, 0)
                sq_pass(1, 1)
                # pre-touch Sqrt while scalar is idle: loads its activation
                # table now instead of on the post-AllGather critical path
                # (the ag-back half-DMA below blocks the scalar queue until
                # the collective completes)
                twarm = vtile("sqrt_warm")
                nc.scalar.activation(twarm[:], sqx[1][:, 1:2], AF.Sqrt)
                for v in range(2):
                    nc.scalar.dma_start(wT[v][:], wq_d[v].ap()[:])
                # vector consumes in landing order
                bn_pass(0, 0)
                sum_pass(0, 0)
                bn_pass(0, 1)
                sum_pass(0, 1)
                bn_pass(1, 0)
                sum_pass(1, 0)
                bn_pass(1, 1)
                sum_pass(1, 1)

            # param loads (tiny) early; the 1.2MB weight-code loads are
            # deferred below until the stats Squares finish, so the x stream
            # that gates the stat-sync trigger gets exclusive HBM bandwidth
            # (weights are not needed until the first matmul, ~45us later)
            for v in range(2):
                for c in range(2):
                    nc.scalar.dma_start(gvec[v][:, c:c + 1],
                                        g_d[v].ap()[c * P:(c + 1) * P])
                    nc.scalar.dma_start(bvec[v][:, c:c + 1],
                                        b_d[v].ap()[c * P:(c + 1) * P])
            nc.scalar.dma_start(hc[:], hc_d.ap()[:])

            # ---------- act pad-row zeroing (vector, overlaps x DMA) --------
            with nc.named_scope("memset"):
                for t in (act0, act1):
                    for r in (0, 57, 58, 115):
                        nc.vector.memset(t[:, r, :, 0:H], 0.0)

            with nc.named_scope("stats0c"):
                for c in range(2):
                    mv = stp.tile([P, 2], F32, name=f"mv0_{c}")
                    nc.vector.bn_aggr(mv[:], xbn[c][:, 0:8, :])
                    sv_ = vtile(f"sv_{c}")
                    nc.vector.scalar_tensor_tensor(
                        sv_[:], mv[:, 0:1], 3136.0, gsx[c][:, 0:1],
                        OP.mult, OP.add)
                    nc.vector.tensor_add(pk[0][:, c:c + 1], sv_[:],
                                         gsx[c][:, 1:2])
                    m2 = vtile(f"xm2_{c}")
                    nc.vector.tensor_mul(m2[:], mv[:, 0:1], mv[:, 0:1])
                    vp = vtile(f"xvp_{c}")
                    nc.vector.tensor_add(vp[:], mv[:, 1:2], m2[:])
                    qv_ = vtile(f"qv_{c}")
                    nc.vector.scalar_tensor_tensor(
                        qv_[:], vp[:], 3136.0, sqx[c][:, 0:1],
                        OP.mult, OP.add)
                    nc.vector.tensor_add(pk[0][:, 2 + c:3 + c], qv_[:],
                                         sqx[c][:, 1:2])
                nc.sync.dma_start(ar_in[0][:], pk[0][:])
                nc.gpsimd.collective_compute(
                    "AllGather", OP.bypass,
                    replica_groups=[list(range(N_CORES))],
                    ins=[ar_in[0].opt()], outs=[ar_out[0].opt()])
                # gather-back split across two queues: the [P,8,4] view is
                # 16B-per-descriptor strided, so halving it in parallel
                # roughly halves the flight time
                nc.sync.dma_start(
                    ag_sb[0][:, 0:4, :],
                    ar_out[0].rearrange("r p c -> p r c")[:, 0:4, :])
                nc.scalar.dma_start(
                    ag_sb[0][:, 4:8, :],
                    ar_out[0].rearrange("r p c -> p r c")[:, 4:8, :])
                nc.vector.tensor_reduce(
                    gpk[0][:], ag_sb[0].rearrange("p r c -> p c r")[:],
                    AX.X, OP.add)

            # ---------- BN coefficients: z = a*S + b on [P, 2] ----------
            # a = 15*gamma*rsqrt(var + eps), b = 15*beta - mean*a.
            # For v=1 the stats are raw integer-S stats and eps_bias = eps'
            # (scale folded in on the host), so the chain is identical.
            def bn_coeffs(v, eps_bias):
                me = vtile(f"me{v}", 4)
                nc.vector.tensor_scalar(me[:], gpk[v][:, 0:4], 1.0 / CNT,
                                        None, OP.mult)
                mean, ex2 = me[:, 0:2], me[:, 2:4]
                m2 = vtile(f"m2{v}", 2)
                nc.vector.tensor_mul(m2[:], mean, mean)
                vpe = vtile(f"vp{v}", 2)
                nc.vector.tensor_sub(vpe[:], ex2, m2[:])
                y = vtile(f"y{v}", 2)
                nc.scalar.activation(y[:], vpe[:], AF.Sqrt, bias=eps_bias)
                rr = vtile(f"rr{v}", 2)
                nc.vector.reciprocal(rr[:], y[:])
                grs = vtile(f"gr{v}", 2)
                nc.vector.tensor_mul(grs[:], gvec[v][:], rr[:])
                nc.vector.tensor_scalar(avec[v][:], grs[:], 15.0, None,
                                        OP.mult)
                mg = vtile(f"mg{v}", 2)
                nc.vector.tensor_mul(mg[:], mean, avec[v][:])
                nc.vector.scalar_tensor_tensor(
                    bbvec[v][:], bvec[v][:], 15.0, mg[:],
                    OP.mult, OP.subtract)

            with nc.named_scope("coeffs0"):
                bn_coeffs(0, hc[:, 0:1])

            # ---------- quantize pipeline ----------
            # stage1: int8(relu(a*x+b)) on scalar (RNE cast rounds)
            # stage2: vector min(.,15) -> fp8 act codes
            def quantize_block(src_t, act_t, c, img, h0, nr, v, names):
                lr = img * 58 + 1 + h0   # logical row
                src_ch = src_t[:, img, h0:h0 + nr, :]
                u = qtp.tile([P, nr, H], I8, name=names + "u", tag="qu",
                             bufs=4, padded_shape=[P, 28, H])
                nc.scalar.activation(u[:], src_ch, AF.Relu,
                                     bias=bbvec[v][:, c:c + 1],
                                     scale=avec[v][:, c:c + 1])
                nc.vector.tensor_scalar(act_t[:, lr:lr + nr, c, 0:H], u[:],
                                        15.0, None, OP.min)

            def quant_phase(src, act_t, v, tag):
                # first block = exactly the 11 input rows of conv window 1,
                # so the first matmul fires as soon as possible after the
                # AllReduce
                for c in range(2):
                    quantize_block(src[c], act_t, c, 0, 0, 11, v,
                                   f"{tag}_{c}00a")
                for c in range(2):
                    quantize_block(src[c], act_t, c, 0, 11, 17, v,
                                   f"{tag}_{c}00b")
                for img in range(IMG):
                    for blk in range(2):
                        if img == 0 and blk == 0:
                            continue
                        for c in range(2):
                            quantize_block(src[c], act_t, c, img, blk * 28,
                                           28, v, f"{tag}_{c}{img}{blk}")

            with nc.named_scope("quant0"):
                quant_phase(x_sb, act0, 0, "q0")

            # ---------- conv (shared), fp8 DoubleRow, K=256 per matmul -------
            def conv(v, epilogue):
                av = actv[v]
                for gi, grp in enumerate(GROUPS):
                    for co in range(2):
                        psums = []
                        for wi, (r0, nr) in enumerate(grp):
                            ps = psp.tile([P, nr, H], F32,
                                          name=f"ps{v}_{gi}_{co}_{wi}",
                                          tag="psw", padded_shape=[P, 9, H])
                            psums.append(ps)
                        for ti, (dy, dx) in enumerate(TAPS):
                            tap = dy * 3 + dx
                            wlo, whi = max(0, 1 - dx), min(H, H + 1 - dx)
                            jlo = max(0, dx - 1)
                            lhsT = wT[v][:, tap, :, co * P:(co + 1) * P]
                            first = ti == 0
                            last = ti == 8
                            for wi, (r0, nr) in enumerate(grp):
                                rows = slice(r0 + dy - 1, r0 + dy - 1 + nr)
                                rhs = av[:, :, rows, jlo:jlo + whi - wlo]
                                if dx == 1:
                                    out = psums[wi][:, :, :]
                                else:
                                    out = psums[wi][:, :, wlo:whi]
                                nc.tensor.matmul(out, lhsT, rhs,
                                                 start=first, stop=last,
                                                 perf_mode=PM.DoubleRow)
                        for wi, (r0, nr) in enumerate(grp):
                            epilogue(co, r0, nr, psums[wi])

            # ---------- conv0 epilogue: S -> SBUF + interior sums ----------
            # sum on scalar (psum drain w/ accum), square-sum on gpsimd
            slot_idx = [0, 0]

            def epi0(co, r0, nr, ps):
                psf = ps.rearrange("p r c -> p (r c)")
                for (rl, n, img, h0) in _runs(r0, nr):
                    sl = psf[:, (rl - r0) * H:(rl - r0 + n) * H]
                    dst = s_fl[co][:, (img * H + h0) * H:(img * H + h0 + n) * H]
                    k = slot_idx[co]
                    slot_idx[co] += 1
                    nc.scalar.activation(dst, sl, AF.Identity,
                                         accum_out=sumS[co][:, k:k + 1])
                    sq = runp.tile([P, n * H], F32, name=f"sq_{co}_{rl}",
                                   tag="sq", bufs=2, padded_shape=[P, 9 * H])
                    nc.vector.scalar_tensor_tensor(
                        sq[:], dst, 0.0, dst, OP.bypass, OP.mult,
                        accum_out=sqS[co][:, k:k + 1])

            with nc.named_scope("conv0"):
                conv(0, epi0)

            # ---------- BN1 stats + AR ----------
            with nc.named_scope("stats1"):
                for c in range(2):
                    ns = slot_idx[c]
                    nc.vector.tensor_reduce(pk[1][:, c:c + 1],
                                            sumS[c][:, 0:ns], AX.X, OP.add)
                    nc.vector.tensor_reduce(pk[1][:, 2 + c:3 + c],
                                            sqS[c][:, 0:ns], AX.X, OP.add)
                nc.sync.dma_start(ar_in[1][:], pk[1][:])
                nc.gpsimd.collective_compute(
                    "AllGather", OP.bypass,
                    replica_groups=[list(range(N_CORES))],
                    ins=[ar_in[1].opt()], outs=[ar_out[1].opt()])
                # gather-back split across two queues: the [P,8,4] view is
                # 16B-per-descriptor strided, so halving it in parallel
                # roughly halves the flight time
                nc.sync.dma_start(
                    ag_sb[1][:, 0:4, :],
                    ar_out[1].rearrange("r p c -> p r c")[:, 0:4, :])
                nc.scalar.dma_start(
                    ag_sb[1][:, 4:8, :],
                    ar_out[1].rearrange("r p c -> p r c")[:, 4:8, :])
                nc.vector.tensor_reduce(
                    gpk[1][:], ag_sb[1].rearrange("p r c -> p c r")[:],
                    AX.X, OP.add)

            with nc.named_scope("coeffs1"):
                bn_coeffs(1, hc[:, 1:2])

            # ---------- quantize1: S -> act1 codes ----------
            with nc.named_scope("quant1"):
                quant_phase(s_sb, act1, 1, "q1")

            # ---------- conv1 + residual epilogue ----------
            def epi1(co, r0, nr, ps):
                psf = ps.rearrange("p r c -> p (r c)")
                for (rl, n, img, h0) in _runs(r0, nr):
                    sl = psf[:, (rl - r0) * H:(rl - r0 + n) * H]
                    xt = x_fl[co][:, (img * H + h0) * H:(img * H + h0 + n) * H]
                    ot = runp.tile([P, n * H], F32, name=f"o_{co}_{rl}",
                                   tag="orun", bufs=5, padded_shape=[P, 9 * H])
                    nc.vector.scalar_tensor_tensor(
                        ot[:], sl, hc[:, 2:3], xt, OP.mult, OP.add)
                    nc.sync.dma_start(
                        ov[co * P:(co + 1) * P, img, h0:h0 + n, :],
                        ot.rearrange("p (a b) -> p a b", b=H)[:])

            with nc.named_scope("conv1"):
                conv(1, epi1)

    nc.compile()
    return nc


def _quantize_weight_host(w):
    """DoReFa 4-bit weight codes on host: returns ([P, 9, 2, 256] fp8 odd-int
    codes, scale M) with w_q = (M/15) * code."""
    t = np.tanh(np.asarray(w, np.float32))
    m = float(np.max(np.abs(t)))
    r = np.rint((7.5 / m) * t + 7.5)           # round(15 * (t/2M + 0.5)), RNE
    code = (2.0 * r - 15.0).astype(np.float32)  # odd ints in [-15, 15]
    # [o, i, kh, kw] -> [i, kh, kw, o] -> [ki, p, tap, o] -> [p, tap, ki, o]
    c2 = code.transpose(1, 2, 3, 0).reshape(2, P, 9, 256)
    wq = np.ascontiguousarray(c2.transpose(1, 2, 0, 3))
    return wq.astype(ml_dtypes.float8_e4m3), m


def _install_ntff_hook():
    """Provide antenv.axon_hooks (absent in this image) via ctypes so that
    run_bass_kernel_spmd(trace=True) can capture NTFF profiles."""
    try:
        from antenv.axon_hooks import get_axon_ntff_profile_hook  # noqa: F401
        return
    except ImportError:
        pass
    import contextlib
    import ctypes
    import types

    so_path = "/opt/axon/libaxon_pjrt.so"
    if not os.path.exists(so_path):
        return
    lib = ctypes.CDLL(so_path)
    if not hasattr(lib, "axon_start_nrt_profile"):
        return
    lib.axon_start_nrt_profile.argtypes = [ctypes.POINTER(ctypes.c_int64),
                                           ctypes.c_size_t]
    lib.axon_start_nrt_profile.restype = ctypes.c_int64
    lib.axon_stop_nrt_profile.argtypes = [ctypes.c_char_p]
    lib.axon_stop_nrt_profile.restype = ctypes.c_int64

    @contextlib.contextmanager
    def _hook(output_dir, device_ids):
        import jax
        jax.devices()
        if device_ids:
            ids = (ctypes.c_int64 * len(device_ids))(*device_ids)
            rc = lib.axon_start_nrt_profile(ids, len(device_ids))
        else:
            rc = lib.axon_start_nrt_profile(None, 0)
        if rc != 0:
            raise RuntimeError(f"axon_start_nrt_profile rc={rc}")
        try:
            yield
        finally:
            n = lib.axon_stop_nrt_profile(str(output_dir).encode())
            print(f"ntff profile: {n} file(s) written to {output_dir}")

    hook_holder = [_hook]
    mod = types.ModuleType("antenv.axon_hooks")
    mod.get_axon_ntff_profile_hook = lambda: hook_holder[0]
    mod.set_axon_ntff_profile_hook = lambda h: hook_holder.__setitem__(0, h)
    import antenv
    sys.modules["antenv.axon_hooks"] = mod
    antenv.axon_hooks = mod


_NC = None


def _get_nc():
    global _NC
    if _NC is None:
        _NC = build()
    return _NC


LAST_RESULTS = None


def kernel(x, bn0_gamma, bn0_beta, conv0_w, bn1_gamma, bn1_beta, conv1_w):
    global LAST_RESULTS
    nc = _get_nc()
    wq0, m0 = _quantize_weight_host(conv0_w)
    wq1, m1 = _quantize_weight_host(conv1_w)
    s0, s1 = m0 / 225.0, m1 / 225.0
    hconst = np.tile(np.array([[EPS, EPS / (s0 * s0), s1]], np.float32),
                     (P, 1))
    shared = {
        "wq0": wq0,
        "wq1": wq1,
        "hconst": hconst,
        "bn0_gamma": np.ascontiguousarray(bn0_gamma, np.float32),
        "bn0_beta": np.ascontiguousarray(bn0_beta, np.float32),
        "bn1_gamma": np.ascontiguousarray(bn1_gamma, np.float32),
        "bn1_beta": np.ascontiguousarray(bn1_beta, np.float32),
    }
    x = np.ascontiguousarray(x, np.float32)
    in_maps = [{"x": x[2 * c:2 * c + 2], **shared} for c in range(N_CORES)]
    trace = bool(int(os.environ.get("KERNEL_TRACE", "0")))
    if trace:
        _install_ntff_hook()
    res = bass_utils.run_bass_kernel_spmd(
        nc, in_maps, core_ids=list(range(N_CORES)), trace=trace)
    LAST_RESULTS = res
    return np.concatenate([res.results[c]["out"] for c in range(N_CORES)], axis=0)
